# revision 19
# baseline (speedup 1.0000x reference)
"""Trainium2 Bass kernel for nn_DiagonalTraining (ragged per-anti-diagonal linear).

Math (reference): for each batch image x[b] (SxS) and each anti-diagonal
i (elements x[b, r, i-r], r=0..i), apply a per-diagonal linear layer:
  out[b,i,q] = sum_{r<=i} x[b,r,i-r] * W[i,q,r] + bias[i,q]   (q <= i)
and scatter back: y[b,q,i-q] = out[b,i,q]; positions with r+c >= S keep x.

Distribution: diagonal i -> core i%8, slot j=i//8 (64 slots per core,
balanced by construction). Host packs, per (core, slot), an augmented
matrix whose rows are the contraction axis r:
  [ D^T | V ]  with D^T[r,b]=x[b,r,i-r], V[r,q]=W[i,q,r]  (r,q < ni=i+1)
zero-padded to a core-independent size NJ=8*(j+1) (>= ni for every
core) so the SPMD program is identical on all cores. The per-diagonal
bias is added on the host while scattering results back (elementwise,
~0.05% of the FLOPs; the whole einsum runs on device).

Device: slots are split into row-chunks (contraction tiles, <=128 rows)
and the chunk rectangles are packed TIGHTLY into [128, WF] window
tiles: full 128-row chunks get their own column strip; short remainder
chunks are paired (within a 16-slot band, heights 8(m+1) + 8(15-m) =
128) and stacked vertically at different partition offsets, so <1% of
shipped bytes are padding. Operands are bf16 (PSUM accumulates f32),
halving HBM traffic vs f32; outputs are stored as bf16 too and
upcast + bias-added on the host. Windows are all resident in SBUF
(12.6 MB < 24 MB); all window loads are issued up front on the gpsimd
SWDGE queue in consumption order, so compute streams right behind the
DMA. Matmuls accumulate psum[32, NJ] per slot inside a bank-packed
4-slot group psum tile; per group one copy (alternating vector/scalar
engines) downcasts psum->bf16 stage tile and a store DMA on the sync
(HWDGE) queue writes it out, overlapping remaining loads.
"""

import sys

for _p in ("/opt/trn_rl_repo", "/opt/pypackages"):
    if _p not in sys.path:
        sys.path.append(_p)

import numpy as np

import concourse.bass as bass  # noqa: F401
import concourse.tile as tile
from concourse import bacc, mybir
from concourse.bass_utils import run_bass_kernel_spmd

B = 32          # batch
S = 512         # seq len / number of diagonals
N_CORES = 8
N_SLOTS = S // N_CORES  # 64 slots per core
DCOL = B        # width of the D^T block (batch on matmul M axis)
GROUP = 2       # slots per psum group
N_GROUPS = N_SLOTS // GROUP
WF = 3072       # window free size (elems per partition)

KCFG = {
    "compute": "bf16",   # "f32" | "f32r" | "bf16"
    "out": "bf16",       # output blob dtype: "f32" | "bf16"
    "psum_bufs": 4,
    "stage_bufs": 4,
    "copy_engines": ("vector", "scalar"),
    "store_engine": "sync",
    # first windows load via the low-latency HWDGE queues while the
    # SWDGE Q7 warms up (~5.7us to first descriptor drain, measured)
    "hwdge_head": 6,
    # max quarters stacked per strip: 4 needs PE tile row position 96,
    # which the rust base_partition path disallows ({0,32,64} only) and
    # no other kernel exercises; 3 keeps pb in {0,32,64}.
    "qstack": 3,
    # "tight": stack 32/64-row units at nonzero partition offsets
    # (explicit tile_position, mixed K per accumulation group).
    # "pad128": every chunk padded to 128 rows at pb=0 (baseline-style
    # uniform K; ~10% more bytes in bf16 than tight).
    "pack": "pad128",
}

# ---- static layout ----------------------------------------------------
# processing order: largest slot first
_ORDER = list(range(N_SLOTS - 1, -1, -1))
_GROUPS = [_ORDER[g * GROUP : (g + 1) * GROUP] for g in range(N_GROUPS)]


def _build_units():
    """Per slot: list of (row_start, unit_rows).

    The PE quantizes operand base partitions (rows<=32 -> pb in
    {0,32,64,96}; rows<=64 -> pb in {0,64}; else pb=0), so chunks come
    in 128/64/32-row units; the last unit of a slot ships zero rows
    beyond NJ (they contribute nothing to the accumulation).
    """
    units = {}
    for j in range(N_SLOTS):
        NJ = 8 * (j + 1)
        f = NJ // 128
        rem = NJ - 128 * f
        us = [(128 * c, 128) for c in range(f)]
        rs = 128 * f
        if rem:
            if KCFG["pack"] == "pad128":
                us.append((rs, 128))
            else:
                q = -(-rem // 32)
                if q == 1:
                    us.append((rs, 32))
                elif q == 2:
                    us.append((rs, 64))
                elif q == 3:
                    us.append((rs, 64))
                    us.append((rs + 64, 32))
                else:
                    us.append((rs, 128))
        units[j] = us
    return units


def _build_strips():
    """Stack units into 128-row strips; returns [(j, row_start, rows)].

    Full units get their own strip. Halves pair with the neighbouring
    slot's half; quarters group 4 consecutive slots' quarters — widths
    differ by 8 between neighbours, so mismatch waste stays tiny.
    Strips are emitted in processing order (largest slot first).
    """
    units = _build_units()
    halves = {}    # j -> (row_start)
    quarters = {}
    for j in range(N_SLOTS):
        for rs, rows in units[j]:
            if rows == 64:
                halves[j] = rs
            elif rows == 32:
                quarters[j] = rs
    # partner tables, built per 16-slot band in descending-j order
    half_groups = {}
    quarter_groups = {}
    for t in range(4):
        hs = [j for j in range(16 * t + 15, 16 * t - 1, -1) if j in halves]
        for a in range(0, len(hs) - 1, 2):
            half_groups[hs[a]] = hs[a : a + 2]
        if len(hs) % 2:
            half_groups[hs[-1]] = [hs[-1]]
        qs = [j for j in range(16 * t + 15, 16 * t - 1, -1) if j in quarters]
        qn = KCFG["qstack"]
        for a in range(0, len(qs), qn):
            quarter_groups[qs[a]] = qs[a : a + qn]
    strips = []
    done = set()
    for j in _ORDER:
        for rs, rows in units[j]:
            if rows == 128:
                strips.append([(j, rs, 128)])
            elif rows == 64:
                if ("h", j) in done:
                    continue
                grp = half_groups.get(j)
                if grp is None:
                    continue
                strips.append([(p, halves[p], 64) for p in grp])
                done.update(("h", p) for p in grp)
            else:
                if ("q", j) in done:
                    continue
                grp = quarter_groups.get(j)
                if grp is None:
                    continue
                strips.append([(p, quarters[p], 32) for p in grp])
                done.update(("q", p) for p in grp)
    return strips


def _wcap(w):
    # small first windows (one/two/four strips of the widest slot) so
    # the first matmuls start early
    return (544, 1088, 2176)[w] if w < 3 else WF


# chunk placement: j -> list of (win, cbase, pbase, rows, row_start)
_SLOT_CHUNKS = {j: [] for j in range(N_SLOTS)}
_WIN_W = []
_cur_win, _cur_col = 0, 0
for _members in _build_strips():
    _sw = max(DCOL + 8 * (_j + 1) for _j, _, _ in _members)
    if _cur_col + _sw > _wcap(_cur_win):
        _WIN_W.append(_cur_col)
        _cur_win += 1
        _cur_col = 0
    _pb = 0
    for _j, _rs, _rows in _members:
        _SLOT_CHUNKS[_j].append((_cur_win, _cur_col, _pb, _rows, _rs))
        _pb += _rows
    _cur_col += _sw
_WIN_W.append(_cur_col)
N_WINS = len(_WIN_W)
for _j in range(N_SLOTS):
    _SLOT_CHUNKS[_j].sort(key=lambda c: c[4])

_WIN_OFF = []
_boff = 0
for _w in range(N_WINS):
    _WIN_OFF.append(_boff)
    _boff += 128 * _WIN_W[_w]
BLOB_ELEMS = _boff

# psum group column layout (bank-aligned, no matmul straddles a bank).
# The psum->stage copies compact the bank-alignment gaps away, so the
# stage/store/output layout uses gap-free "compact" columns.
_BANK = 512
_GROUP_COLS = []    # g -> [(j, psum_col)]
_GROUP_W = []       # g -> psum tile width (with gaps)
_GROUP_RANGES = []  # g -> [(psum_off, compact_off, width)] copy ranges
_GROUP_CW = []      # g -> compact width
for _slots in _GROUPS:
    _col = 0
    _ccol = 0
    _cols = []
    _ranges = []
    for _j in _slots:
        _NJ = 8 * (_j + 1)
        if _col // _BANK != (_col + _NJ - 1) // _BANK:
            _col = ((_col + _BANK - 1) // _BANK) * _BANK
        if _ranges and _ranges[-1][0] + _ranges[-1][2] == _col:
            _ranges[-1] = (_ranges[-1][0], _ranges[-1][1], _ranges[-1][2] + _NJ)
        else:
            _ranges.append((_col, _ccol, _NJ))
        _cols.append((_j, _col))
        _col += _NJ
        _ccol += _NJ
    _GROUP_COLS.append(_cols)
    _GROUP_W.append(_col)
    _GROUP_RANGES.append(_ranges)
    _GROUP_CW.append(_ccol)

_GOUT_OFF = []
_SLOT_OUT = {}      # j -> (g, compact col)
_goff = 0
for _g in range(N_GROUPS):
    _GOUT_OFF.append(_goff)
    _ccol = 0
    for _j, _col in _GROUP_COLS[_g]:
        _SLOT_OUT[_j] = (_g, _ccol)
        _ccol += 8 * (_j + 1)
    _goff += B * _GROUP_CW[_g]
OUT_ELEMS = _goff

_compiled_nc = None


def _dt(nc, name):
    f32 = mybir.dt.float32
    return {
        "f32": f32,
        "f32r": mybir.dt.float32r,
        "bf16": mybir.dt.bfloat16,
    }[name]


def _build_program():
    global _compiled_nc
    if _compiled_nc is not None:
        return _compiled_nc

    from contextlib import ExitStack

    nc = bacc.Bacc("TRN2", target_bir_lowering=False, debug=False)
    f32 = mybir.dt.float32
    mm_dt = _dt(nc, KCFG["compute"])
    out_dt = _dt(nc, KCFG["out"])
    blob = nc.dram_tensor("blob", [BLOB_ELEMS], mm_dt, kind="ExternalInput").ap()
    outb = nc.dram_tensor("outblob", [OUT_ELEMS], out_dt, kind="ExternalOutput").ap()

    store_eng = getattr(nc, KCFG["store_engine"])
    copy_engs = [getattr(nc, e) for e in KCFG["copy_engines"]]

    with tile.TileContext(nc) as tc, ExitStack() as ctx:
        win_pool = ctx.enter_context(tc.tile_pool(name="win", bufs=1))
        stage_pool = ctx.enter_context(
            tc.tile_pool(name="stage", bufs=KCFG["stage_bufs"])
        )
        psum_pool = ctx.enter_context(
            tc.tile_pool(name="psum", bufs=KCFG["psum_bufs"], space="PSUM")
        )

        # all windows resident; issue every load up front in order.
        # The first hwdge_head windows go on the sync/scalar HWDGE
        # queues (low latency), the rest stream on the gpsimd SWDGE.
        nh = KCFG["hwdge_head"]
        win_tiles = []
        for w in range(N_WINS):
            wf = _WIN_W[w]
            t = win_pool.tile([128, wf], mm_dt, name=f"win{w}", tag=f"win{w}")
            src = blob[_WIN_OFF[w] : _WIN_OFF[w] + 128 * wf].rearrange(
                "(p f) -> p f", p=128, f=wf
            )
            if w < nh:
                eng = nc.sync if w % 2 == 0 else nc.scalar
            else:
                eng = nc.gpsimd
            eng.dma_start(t[:], src)
            win_tiles.append(t)

        for g, slots in enumerate(_GROUPS):
            gw = _GROUP_W[g]
            psum_t = psum_pool.tile([B, gw], f32, name=f"psum{g}", tag="psum")
            for j, col in _GROUP_COLS[g]:
                NJ = 8 * (j + 1)
                wd = DCOL + NJ
                chs = _SLOT_CHUNKS[j]
                for c, (w, cb, pb, rows, _rs) in enumerate(chs):
                    t = win_tiles[w]
                    nc.tensor.matmul(
                        psum_t[:, col : col + NJ],
                        t[pb : pb + rows, cb : cb + DCOL],
                        t[pb : pb + rows, cb + DCOL : cb + wd],
                        start=(c == 0),
                        stop=(c == len(chs) - 1),
                        tile_position=None if pb == 0 else (pb, 0),
                    )
            cw = _GROUP_CW[g]
            stage_t = stage_pool.tile([B, cw], out_dt, name=f"st{g}", tag="stage")
            # split every copy range across both copy engines so the
            # psum buffer frees twice as fast (the tail is copy-paced)
            def _copy(eng, dst_ap, src_ap):
                if eng is nc.scalar:
                    eng.copy(dst_ap, src_ap)
                else:
                    eng.tensor_copy(dst_ap, src_ap)

            for po, co, wdt in _GROUP_RANGES[g]:
                h = (wdt // 2) if wdt >= 256 else wdt
                _copy(
                    copy_engs[0],
                    stage_t[:, co : co + h],
                    psum_t[:, po : po + h],
                )
                if h < wdt:
                    _copy(
                        copy_engs[1],
                        stage_t[:, co + h : co + wdt],
                        psum_t[:, po + h : po + wdt],
                    )
            dst = outb[_GOUT_OFF[g] : _GOUT_OFF[g] + B * cw].rearrange(
                "(p w) -> p w", p=B, w=cw
            )
            store_eng.dma_start(dst, stage_t[:])

    nc.compile()
    _compiled_nc = nc
    return nc


def _np_dt(name):
    if name == "bf16":
        import ml_dtypes

        return ml_dtypes.bfloat16
    return np.float32


def _pack_core(k, x, W, bias):
    np_dt = _np_dt(KCFG["compute"])
    blob = np.zeros(BLOB_ELEMS, np_dt)
    for j in range(N_SLOTS):
        i = N_CORES * j + k
        ni = i + 1
        NJ = 8 * (j + 1)
        wd = DCOL + NJ
        M = np.zeros((NJ, wd), np.float32)
        r = np.arange(ni)
        M[:ni, :DCOL] = x[:, r, i - r].T               # D^T[r, b]
        M[:ni, DCOL : DCOL + ni] = W[i, :ni, :ni].T    # V[r, q]
        for w, cb, pb, rows, rs in _SLOT_CHUNKS[j]:
            rl = M[rs : rs + rows]          # may be shorter than rows
            wf = _WIN_W[w]
            img = blob[_WIN_OFF[w] : _WIN_OFF[w] + 128 * wf].reshape(128, wf)
            img[pb : pb + rl.shape[0], cb : cb + wd] = rl.astype(np_dt)
    return blob


def kernel(x, W, b):
    x = np.asarray(x, np.float32)
    W = np.asarray(W, np.float32)
    b = np.asarray(b, np.float32)

    nc = _build_program()
    in_maps = [{"blob": _pack_core(k, x, W, b)} for k in range(N_CORES)]
    res = run_bass_kernel_spmd(nc, in_maps, list(range(N_CORES)))

    y = x.copy()
    for k in range(N_CORES):
        ob = res.results[k]["outblob"]
        for j in range(N_SLOTS):
            i = N_CORES * j + k
            ni = i + 1
            g, col = _SLOT_OUT[j]
            cw = _GROUP_CW[g]
            blk = np.asarray(
                ob[_GOUT_OFF[g] : _GOUT_OFF[g] + B * cw], np.float32
            ).reshape(B, cw)
            q = np.arange(ni)
            y[:, q, i - q] = blk[:, col : col + ni] + b[i, :ni][None]
    return y


# revision 20
# speedup vs baseline: 1.0549x; 1.0549x over previous
"""Trainium2 Bass kernel for nn_DiagonalTraining (ragged per-anti-diagonal linear).

Math (reference): for each batch image x[b] (SxS) and each anti-diagonal
i (elements x[b, r, i-r], r=0..i), apply a per-diagonal linear layer:
  out[b,i,q] = sum_{r<=i} x[b,r,i-r] * W[i,q,r] + bias[i,q]   (q <= i)
and scatter back: y[b,q,i-q] = out[b,i,q]; positions with r+c >= S keep x.

Distribution: diagonal i -> core i%8, slot j=i//8 (64 slots per core,
balanced by construction). Host packs, per (core, slot), an augmented
matrix whose rows are the contraction axis r:
  [ D^T | V ]  with D^T[r,b]=x[b,r,i-r], V[r,q]=W[i,q,r]  (r,q < ni=i+1)
zero-padded to a core-independent size NJ=8*(j+1) (>= ni for every
core) so the SPMD program is identical on all cores. The per-diagonal
bias is added on the host while scattering results back (elementwise,
~0.05% of the FLOPs; the whole einsum runs on device).

Device: slots are split into row-chunks (contraction tiles, <=128 rows)
and the chunk rectangles are packed TIGHTLY into [128, WF] window
tiles: full 128-row chunks get their own column strip; short remainder
chunks are paired (within a 16-slot band, heights 8(m+1) + 8(15-m) =
128) and stacked vertically at different partition offsets, so <1% of
shipped bytes are padding. Operands are bf16 (PSUM accumulates f32),
halving HBM traffic vs f32; outputs are stored as bf16 too and
upcast + bias-added on the host. Windows are all resident in SBUF
(12.6 MB < 24 MB); all window loads are issued up front on the gpsimd
SWDGE queue in consumption order, so compute streams right behind the
DMA. Matmuls accumulate psum[32, NJ] per slot inside a bank-packed
4-slot group psum tile; per group one copy (alternating vector/scalar
engines) downcasts psum->bf16 stage tile and a store DMA on the sync
(HWDGE) queue writes it out, overlapping remaining loads.
"""

import sys

for _p in ("/opt/trn_rl_repo", "/opt/pypackages"):
    if _p not in sys.path:
        sys.path.append(_p)

import numpy as np

import concourse.bass as bass  # noqa: F401
import concourse.tile as tile
from concourse import bacc, mybir
from concourse.bass_utils import run_bass_kernel_spmd

B = 32          # batch
S = 512         # seq len / number of diagonals
N_CORES = 8
N_SLOTS = S // N_CORES  # 64 slots per core
DCOL = B        # width of the D^T block (batch on matmul M axis)
GROUP = 4       # slots per psum group
N_GROUPS = N_SLOTS // GROUP
WF = 3072       # window free size (elems per partition)

KCFG = {
    "compute": "bf16",   # "f32" | "f32r" | "bf16"
    "out": "bf16",       # output blob dtype: "f32" | "bf16"
    "psum_bufs": 2,
    "stage_bufs": 3,
    "copy_engines": ("vector", "scalar"),
    "store_engine": "sync",
    # first windows load via the low-latency HWDGE queues while the
    # SWDGE Q7 warms up (~5.7us to first descriptor drain, measured)
    "hwdge_head": 0,
    # max quarters stacked per strip: 4 needs PE tile row position 96,
    # which the rust base_partition path disallows ({0,32,64} only) and
    # no other kernel exercises; 3 keeps pb in {0,32,64}.
    "qstack": 3,
    # "tight": stack 32/64-row units at nonzero partition offsets
    # (explicit tile_position, mixed K per accumulation group).
    # "pad128": every chunk padded to 128 rows at pb=0 (baseline-style
    # uniform K; ~10% more bytes in bf16 than tight).
    "pack": "pad128",
}

# ---- static layout ----------------------------------------------------
# processing order: largest slot first
_ORDER = list(range(N_SLOTS - 1, -1, -1))
_GROUPS = [_ORDER[g * GROUP : (g + 1) * GROUP] for g in range(N_GROUPS)]


def _build_units():
    """Per slot: list of (row_start, unit_rows).

    The PE quantizes operand base partitions (rows<=32 -> pb in
    {0,32,64,96}; rows<=64 -> pb in {0,64}; else pb=0), so chunks come
    in 128/64/32-row units; the last unit of a slot ships zero rows
    beyond NJ (they contribute nothing to the accumulation).
    """
    units = {}
    for j in range(N_SLOTS):
        NJ = 8 * (j + 1)
        f = NJ // 128
        rem = NJ - 128 * f
        us = [(128 * c, 128) for c in range(f)]
        rs = 128 * f
        if rem:
            if KCFG["pack"] == "pad128":
                us.append((rs, 128))
            else:
                q = -(-rem // 32)
                if q == 1:
                    us.append((rs, 32))
                elif q == 2:
                    us.append((rs, 64))
                elif q == 3:
                    us.append((rs, 64))
                    us.append((rs + 64, 32))
                else:
                    us.append((rs, 128))
        units[j] = us
    return units


def _build_strips():
    """Stack units into 128-row strips; returns [(j, row_start, rows)].

    Full units get their own strip. Halves pair with the neighbouring
    slot's half; quarters group 4 consecutive slots' quarters — widths
    differ by 8 between neighbours, so mismatch waste stays tiny.
    Strips are emitted in processing order (largest slot first).
    """
    units = _build_units()
    halves = {}    # j -> (row_start)
    quarters = {}
    for j in range(N_SLOTS):
        for rs, rows in units[j]:
            if rows == 64:
                halves[j] = rs
            elif rows == 32:
                quarters[j] = rs
    # partner tables, built per 16-slot band in descending-j order
    half_groups = {}
    quarter_groups = {}
    for t in range(4):
        hs = [j for j in range(16 * t + 15, 16 * t - 1, -1) if j in halves]
        for a in range(0, len(hs) - 1, 2):
            half_groups[hs[a]] = hs[a : a + 2]
        if len(hs) % 2:
            half_groups[hs[-1]] = [hs[-1]]
        qs = [j for j in range(16 * t + 15, 16 * t - 1, -1) if j in quarters]
        qn = KCFG["qstack"]
        for a in range(0, len(qs), qn):
            quarter_groups[qs[a]] = qs[a : a + qn]
    strips = []
    done = set()
    for j in _ORDER:
        for rs, rows in units[j]:
            if rows == 128:
                strips.append([(j, rs, 128)])
            elif rows == 64:
                if ("h", j) in done:
                    continue
                grp = half_groups.get(j)
                if grp is None:
                    continue
                strips.append([(p, halves[p], 64) for p in grp])
                done.update(("h", p) for p in grp)
            else:
                if ("q", j) in done:
                    continue
                grp = quarter_groups.get(j)
                if grp is None:
                    continue
                strips.append([(p, quarters[p], 32) for p in grp])
                done.update(("q", p) for p in grp)
    return strips


def _wcap(w):
    # small first windows (one/two/four strips of the widest slot) so
    # the first matmuls start early
    return (544, 1088, 2176)[w] if w < 3 else WF


# chunk placement: j -> list of (win, cbase, pbase, rows, row_start)
_SLOT_CHUNKS = {j: [] for j in range(N_SLOTS)}
_WIN_W = []
_cur_win, _cur_col = 0, 0
for _members in _build_strips():
    _sw = max(DCOL + 8 * (_j + 1) for _j, _, _ in _members)
    if _cur_col + _sw > _wcap(_cur_win):
        _WIN_W.append(_cur_col)
        _cur_win += 1
        _cur_col = 0
    _pb = 0
    for _j, _rs, _rows in _members:
        _SLOT_CHUNKS[_j].append((_cur_win, _cur_col, _pb, _rows, _rs))
        _pb += _rows
    _cur_col += _sw
_WIN_W.append(_cur_col)
N_WINS = len(_WIN_W)
for _j in range(N_SLOTS):
    _SLOT_CHUNKS[_j].sort(key=lambda c: c[4])

_WIN_OFF = []
_boff = 0
for _w in range(N_WINS):
    _WIN_OFF.append(_boff)
    _boff += 128 * _WIN_W[_w]
BLOB_ELEMS = _boff

# psum group column layout (bank-aligned, no matmul straddles a bank).
# The psum->stage copies compact the bank-alignment gaps away, so the
# stage/store/output layout uses gap-free "compact" columns.
_BANK = 512
_GROUP_COLS = []    # g -> [(j, psum_col)]
_GROUP_W = []       # g -> psum tile width (with gaps)
_GROUP_RANGES = []  # g -> [(psum_off, compact_off, width)] copy ranges
_GROUP_CW = []      # g -> compact width
for _slots in _GROUPS:
    _col = 0
    _ccol = 0
    _cols = []
    _ranges = []
    for _j in _slots:
        _NJ = 8 * (_j + 1)
        if _col // _BANK != (_col + _NJ - 1) // _BANK:
            _col = ((_col + _BANK - 1) // _BANK) * _BANK
        if _ranges and _ranges[-1][0] + _ranges[-1][2] == _col:
            _ranges[-1] = (_ranges[-1][0], _ranges[-1][1], _ranges[-1][2] + _NJ)
        else:
            _ranges.append((_col, _ccol, _NJ))
        _cols.append((_j, _col))
        _col += _NJ
        _ccol += _NJ
    _GROUP_COLS.append(_cols)
    _GROUP_W.append(_col)
    _GROUP_RANGES.append(_ranges)
    _GROUP_CW.append(_ccol)

_GOUT_OFF = []
_SLOT_OUT = {}      # j -> (g, compact col)
_goff = 0
for _g in range(N_GROUPS):
    _GOUT_OFF.append(_goff)
    _ccol = 0
    for _j, _col in _GROUP_COLS[_g]:
        _SLOT_OUT[_j] = (_g, _ccol)
        _ccol += 8 * (_j + 1)
    _goff += B * _GROUP_CW[_g]
OUT_ELEMS = _goff

_compiled_nc = None


def _dt(nc, name):
    f32 = mybir.dt.float32
    return {
        "f32": f32,
        "f32r": mybir.dt.float32r,
        "bf16": mybir.dt.bfloat16,
    }[name]


def _build_program():
    global _compiled_nc
    if _compiled_nc is not None:
        return _compiled_nc

    from contextlib import ExitStack

    nc = bacc.Bacc("TRN2", target_bir_lowering=False, debug=False)
    f32 = mybir.dt.float32
    mm_dt = _dt(nc, KCFG["compute"])
    out_dt = _dt(nc, KCFG["out"])
    blob = nc.dram_tensor("blob", [BLOB_ELEMS], mm_dt, kind="ExternalInput").ap()
    outb = nc.dram_tensor("outblob", [OUT_ELEMS], out_dt, kind="ExternalOutput").ap()

    store_eng = getattr(nc, KCFG["store_engine"])
    copy_engs = [getattr(nc, e) for e in KCFG["copy_engines"]]

    with tile.TileContext(nc) as tc, ExitStack() as ctx:
        win_pool = ctx.enter_context(tc.tile_pool(name="win", bufs=1))
        stage_pool = ctx.enter_context(
            tc.tile_pool(name="stage", bufs=KCFG["stage_bufs"])
        )
        psum_pool = ctx.enter_context(
            tc.tile_pool(name="psum", bufs=KCFG["psum_bufs"], space="PSUM")
        )

        # all windows resident; issue every load up front in order.
        # The first hwdge_head windows go on the sync/scalar HWDGE
        # queues (low latency), the rest stream on the gpsimd SWDGE.
        nh = KCFG["hwdge_head"]
        win_tiles = []
        for w in range(N_WINS):
            wf = _WIN_W[w]
            t = win_pool.tile([128, wf], mm_dt, name=f"win{w}", tag=f"win{w}")
            src = blob[_WIN_OFF[w] : _WIN_OFF[w] + 128 * wf].rearrange(
                "(p f) -> p f", p=128, f=wf
            )
            if w < nh:
                eng = nc.sync if w % 2 == 0 else nc.scalar
            else:
                eng = nc.gpsimd
            eng.dma_start(t[:], src)
            win_tiles.append(t)

        for g, slots in enumerate(_GROUPS):
            gw = _GROUP_W[g]
            psum_t = psum_pool.tile([B, gw], f32, name=f"psum{g}", tag="psum")
            for j, col in _GROUP_COLS[g]:
                NJ = 8 * (j + 1)
                wd = DCOL + NJ
                chs = _SLOT_CHUNKS[j]
                for c, (w, cb, pb, rows, _rs) in enumerate(chs):
                    t = win_tiles[w]
                    nc.tensor.matmul(
                        psum_t[:, col : col + NJ],
                        t[pb : pb + rows, cb : cb + DCOL],
                        t[pb : pb + rows, cb + DCOL : cb + wd],
                        start=(c == 0),
                        stop=(c == len(chs) - 1),
                        tile_position=None if pb == 0 else (pb, 0),
                    )
            cw = _GROUP_CW[g]
            stage_t = stage_pool.tile([B, cw], out_dt, name=f"st{g}", tag="stage")
            # split every copy range across both copy engines so the
            # psum buffer frees twice as fast (the tail is copy-paced)
            def _copy(eng, dst_ap, src_ap):
                if eng is nc.scalar:
                    eng.copy(dst_ap, src_ap)
                else:
                    eng.tensor_copy(dst_ap, src_ap)

            for po, co, wdt in _GROUP_RANGES[g]:
                h = (wdt // 2) if wdt >= 256 else wdt
                _copy(
                    copy_engs[0],
                    stage_t[:, co : co + h],
                    psum_t[:, po : po + h],
                )
                if h < wdt:
                    _copy(
                        copy_engs[1],
                        stage_t[:, co + h : co + wdt],
                        psum_t[:, po + h : po + wdt],
                    )
            dst = outb[_GOUT_OFF[g] : _GOUT_OFF[g] + B * cw].rearrange(
                "(p w) -> p w", p=B, w=cw
            )
            store_eng.dma_start(dst, stage_t[:])

    nc.compile()
    _compiled_nc = nc
    return nc


def _np_dt(name):
    if name == "bf16":
        import ml_dtypes

        return ml_dtypes.bfloat16
    return np.float32


def _pack_core(k, x, W, bias):
    np_dt = _np_dt(KCFG["compute"])
    blob = np.zeros(BLOB_ELEMS, np_dt)
    for j in range(N_SLOTS):
        i = N_CORES * j + k
        ni = i + 1
        NJ = 8 * (j + 1)
        wd = DCOL + NJ
        M = np.zeros((NJ, wd), np.float32)
        r = np.arange(ni)
        M[:ni, :DCOL] = x[:, r, i - r].T               # D^T[r, b]
        M[:ni, DCOL : DCOL + ni] = W[i, :ni, :ni].T    # V[r, q]
        for w, cb, pb, rows, rs in _SLOT_CHUNKS[j]:
            rl = M[rs : rs + rows]          # may be shorter than rows
            wf = _WIN_W[w]
            img = blob[_WIN_OFF[w] : _WIN_OFF[w] + 128 * wf].reshape(128, wf)
            img[pb : pb + rl.shape[0], cb : cb + wd] = rl.astype(np_dt)
    return blob


def kernel(x, W, b):
    x = np.asarray(x, np.float32)
    W = np.asarray(W, np.float32)
    b = np.asarray(b, np.float32)

    nc = _build_program()
    in_maps = [{"blob": _pack_core(k, x, W, b)} for k in range(N_CORES)]
    res = run_bass_kernel_spmd(nc, in_maps, list(range(N_CORES)))

    y = x.copy()
    for k in range(N_CORES):
        ob = res.results[k]["outblob"]
        for j in range(N_SLOTS):
            i = N_CORES * j + k
            ni = i + 1
            g, col = _SLOT_OUT[j]
            cw = _GROUP_CW[g]
            blk = np.asarray(
                ob[_GOUT_OFF[g] : _GOUT_OFF[g] + B * cw], np.float32
            ).reshape(B, cw)
            q = np.arange(ni)
            y[:, q, i - q] = blk[:, col : col + ni] + b[i, :ni][None]
    return y
